# revision 15
# baseline (speedup 1.0000x reference)
"""CRF negative-log-likelihood kernel for 8 Trainium2 NeuronCores.

Strategy (data-parallel over batch, 128 sequences per core):

Denominator (log-partition) — scaled-probability-space scan in bf16:
    E = exp(T - 7*ln2) in bf16.  Forward chain (t = 0..255) and backward
    chain (t = 511..256) run simultaneously, stacked on partitions 0-47
    and 64-111 of a 128-row state (rows 48-63 / 112-127 carry junk that
    the zero rows of the 128x128 block-diagonal W annihilate each step).
    The host pre-interleaves emissions into 128-wide scan steps
    [fwd em_s | pad | bwd em_{511-s} | pad] in bf16, so that after a
    chunked ACT exp, ONE 128x128 bf16 PE transpose per step produces the
    x-tile in exactly the state layout.  Transposes land in 4-step PSUM
    groups and one ACT copy per group moves them to SBUF (DVE cannot
    read two PSUM operands).  Per scan step: one 128x128 bf16 matmul +
    one DVE multiply.  Join at t=256: Z = sum_i (E^T P_255)[i]*R_256[i];
    logZ = ln(Z) + 511*7*ln2 (host adds the constant).

Numerator (gold-path score):
    em-gold: one-hot built with gpsimd.local_scatter on a compact bf16
    copy of the emissions (scatter 1.0 at 48*(t%32) + tag), DVE bf16
    multiply, ACT-accum reduce per 32-step chunk, final DVE reduce.
    transition/start/end-gold: gather from a replicated 2401-entry table
    [T.flat | start | end | 0] with gpsimd.ap_gather (34-int16-wide
    4-byte-aligned index slices per instruction), ACT-accum reduce.

Outputs per core: zlog (1,128) = ln(Z_scaled) and gold (128,2) =
[em_gold, transition+start+end gold].  Host: loss = mean(zlog +
511*7*ln2 - gold0 - gold1).
"""

import math

import numpy as np

B = 128  # batch rows per core
S = 512
NT = 48
BO = 64  # partition offset of the backward chain
H = 128  # state height
NS = S // 2  # 256 scan steps (incl. init)
NCORES = 8
LOG_SCALE = 7 * math.log(2.0)
CH = 64  # interleaved-chunk size (scan steps)
NCH = NS // CH  # 4 interleaved chunks
CC = 128  # compact-chunk size (time steps)
SC = 32  # scatter chunk size (time steps)
GRP = 4  # scan steps per PSUM transpose group
TBL = NT * NT + NT + NT + 1  # 2401-entry gather table
IPG = 34  # gather idxs per partition per instruction (4-byte aligned)
NGI = 16 * IPG  # 544 gather slots per sequence (511 + 2 + 31 pad)

_CACHE = {}


def _build():
    import concourse.bass as bass
    import concourse.bacc as bacc
    import concourse.tile as tile
    from concourse import mybir
    from concourse.masks import make_identity
    from concourse import library_config
    from concourse.tile import add_dep_helper

    f32 = mybir.dt.float32
    bf16 = mybir.dt.bfloat16
    i16 = mybir.dt.int16
    AF = mybir.ActivationFunctionType
    ALU = mybir.AluOpType

    nc = bacc.Bacc("TRN2", target_bir_lowering=False, debug=False)

    emi_d = nc.dram_tensor("emi", (B, NS * H), bf16, kind="ExternalInput").ap()
    emc_d = nc.dram_tensor("emc", (B, S * NT), bf16, kind="ExternalInput").ap()
    sidx_d = nc.dram_tensor("sidx", (B, S), i16, kind="ExternalInput").ap()
    gidx_d = nc.dram_tensor("gidxw", (B, NGI), i16, kind="ExternalInput").ap()
    trans_d = nc.dram_tensor("trans", (NT, NT), f32, kind="ExternalInput").ap()
    start_d = nc.dram_tensor("start_t", (NT,), f32, kind="ExternalInput").ap()
    end_d = nc.dram_tensor("end_t", (NT,), f32, kind="ExternalInput").ap()
    zlog_d = nc.dram_tensor("zlog", (1, B), f32, kind="ExternalOutput").ap()
    gold_d = nc.dram_tensor("gold", (B, 2), f32, kind="ExternalOutput").ap()

    with tile.TileContext(nc) as tc:
        with (
            tc.tile_pool(name="consts", bufs=1) as consts,
            tc.tile_pool(name="emi", bufs=2) as emi_pool,
            tc.tile_pool(name="xi", bufs=2) as xi_pool,
            tc.tile_pool(name="emc", bufs=2) as emc_pool,
            tc.tile_pool(name="x4", bufs=3) as x4_pool,
            tc.tile_pool(name="pst", bufs=3) as pst_pool,
            tc.tile_pool(name="oh", bufs=16) as oh_pool,
            tc.tile_pool(name="num", bufs=2) as num_pool,
            tc.tile_pool(name="gout", bufs=16) as gout_pool,
            tc.tile_pool(name="scr", bufs=1) as scr_pool,
            tc.tile_pool(name="small", bufs=2) as small_pool,
            tc.tile_pool(name="psx", bufs=3, space="PSUM") as psx_pool,
            tc.tile_pool(name="pss", bufs=2, space="PSUM") as pss_pool,
            tc.tile_pool(name="psj", bufs=1, space="PSUM") as psj_pool,
        ):
            # ---------------- constants ----------------
            ident_b = consts.tile([128, 128], bf16)
            make_identity(nc, ident_b)
            ident_f = consts.tile([NT, NT], f32)
            make_identity(nc, ident_f)

            t_sb = consts.tile([NT, NT], f32)
            nc.sync.dma_start(out=t_sb, in_=trans_d)
            # start/end padded to 64 rows (rows 48:64 = 0 -> exp = 1) so
            # the init ACT copies cover the junk rows with finite data.
            start_sb = consts.tile([BO, 1], f32)
            nc.vector.memset(start_sb, 0.0)
            nc.sync.dma_start(out=start_sb[0:NT, :], in_=start_d)
            end_sb = consts.tile([BO, 1], f32)
            nc.vector.memset(end_sb, 0.0)
            nc.sync.dma_start(out=end_sb[0:NT, :], in_=end_d)

            exp_st = consts.tile([BO, 1], f32)
            nc.scalar.activation(exp_st, start_sb, AF.Exp)
            exp_en = consts.tile([BO, 1], f32)
            nc.scalar.activation(exp_en, end_sb, AF.Exp)

            ones48 = consts.tile([NT, 1], f32)
            nc.vector.memset(ones48, 1.0)
            nls = consts.tile([NT, 1], f32)
            nc.vector.memset(nls, -LOG_SCALE)

            # W = blockdiag(E @0, 0, E^T @64), E = exp(T - LOG_SCALE), bf16
            w_sb = consts.tile([H, H], bf16)
            nc.vector.memset(w_sb, 0.0)
            ps_tt = psj_pool.tile([NT, NT], f32)
            nc.tensor.transpose(ps_tt, t_sb, ident_f)
            nc.scalar.activation(w_sb[0:NT, 0:NT], t_sb, AF.Exp, bias=nls[:, 0:1])
            nc.scalar.activation(
                w_sb[BO : BO + NT, BO : BO + NT], ps_tt, AF.Exp, bias=nls[:, 0:1]
            )

            # gather table [T.flat | start | end | 0] replicated on 128 parts
            table = consts.tile([B, TBL], f32)
            nc.sync.dma_start(
                out=table[:, 0 : NT * NT],
                in_=bass.AP(
                    tensor=trans_d.tensor,
                    offset=trans_d.offset,
                    ap=[[0, B], [1, NT * NT]],
                ),
            )
            nc.sync.dma_start(
                out=table[:, NT * NT : NT * NT + NT],
                in_=bass.AP(
                    tensor=start_d.tensor,
                    offset=start_d.offset,
                    ap=[[0, B], [1, NT]],
                ),
            )
            nc.sync.dma_start(
                out=table[:, NT * NT + NT : NT * NT + 2 * NT],
                in_=bass.AP(
                    tensor=end_d.tensor,
                    offset=end_d.offset,
                    ap=[[0, B], [1, NT]],
                ),
            )
            nc.vector.memset(table[:, TBL - 1 : TBL], 0.0)

            data_ones = consts.tile([B, SC], bf16)
            nc.vector.memset(data_ones, 1.0)

            idx16 = consts.tile([B, S], i16)
            nc.sync.dma_start(out=idx16, in_=sidx_d)
            gidx16 = consts.tile([B, NGI], i16)
            nc.sync.dma_start(out=gidx16, in_=gidx_d)

            # ---------------- chunked loads + exp ----------------
            xi_tiles = {}
            for c in range(NCH):
                emt = emi_pool.tile([B, CH * H], bf16, tag="emi")
                nc.sync.dma_start(
                    out=emt, in_=emi_d[:, H * CH * c : H * CH * (c + 1)]
                )
                xt = xi_pool.tile([B, CH * H], bf16, tag="xi")
                nc.scalar.activation(xt, emt, AF.Exp)
                xi_tiles[c] = xt

            emc_tiles = {}
            for c in range(S // CC):
                emt = emc_pool.tile([B, CC * NT], bf16, tag="emc")
                nc.sync.dma_start(
                    out=emt, in_=emc_d[:, NT * CC * c : NT * CC * (c + 1)]
                )
                emc_tiles[c] = emt

            # ---------------- numerator: scatters (gpsimd, early) --------
            ld_ls = nc.gpsimd.load_library(library_config.local_scatter)
            scatter_insts = []
            oh_tiles = []
            for k in range(S // SC):
                oh = oh_pool.tile([B, SC * NT], bf16, tag="oh")
                sc_i = nc.gpsimd.local_scatter(
                    out_ap=oh,
                    data_ap=data_ones,
                    idxs_ap=idx16[:, SC * k : SC * (k + 1)],
                    channels=B,
                    num_elems=SC * NT,
                    num_idxs=SC,
                )
                add_dep_helper(sc_i.ins, ld_ls.ins, reason="lib order")
                scatter_insts.append(sc_i)
                oh_tiles.append(oh)

            ld_ag = nc.gpsimd.load_library(library_config.ap_gather)
            for sc_i in scatter_insts:
                add_dep_helper(ld_ag.ins, sc_i.ins, reason="lib order")

            reds_em = consts.tile([B, S // SC], f32)
            reds_tr = consts.tile([B, 16], f32)

            def em_gold_piece(k):
                """DVE mul + ACT accum-reduce for scatter chunk k."""
                c, hh = k // 4, k % 4
                prod = num_pool.tile([B, SC * NT], bf16, tag="prod")
                nc.vector.tensor_mul(
                    prod,
                    emc_tiles[c][:, SC * NT * hh : SC * NT * (hh + 1)],
                    oh_tiles[k],
                )
                scr = scr_pool.tile([B, SC * NT], bf16, tag="scr")
                nc.scalar.activation(
                    scr, prod, AF.Copy, accum_out=reds_em[:, k : k + 1]
                )

            # all 16 gathers issued up front so the gpsimd queue runs
            # [scatters, lib, gathers] before any tile-release bookkeeping
            g_out_tiles = []
            for i in range(16):
                g_out = gout_pool.tile([B, NGI], f32, tag="gout")
                ag_i = nc.gpsimd.ap_gather(
                    out_ap=g_out,
                    in_ap=table,
                    idxs_ap=gidx16[:, IPG * i : IPG * (i + 1)],
                    channels=B,
                    num_elems=TBL,
                    d=1,
                    num_idxs=NGI,
                )
                add_dep_helper(ag_i.ins, ld_ag.ins, reason="lib order")
                g_out_tiles.append(g_out)

            def gather_piece(i):
                nc.vector.tensor_reduce(
                    out=reds_tr[:, i : i + 1],
                    in_=g_out_tiles[i],
                    axis=mybir.AxisListType.X,
                    op=ALU.add,
                )

            # ---------------- scan ----------------
            def xpose_group(g):
                """Transpose x for scan steps 4g..4g+3 into one PSUM tile,
                then one ACT copy to SBUF bf16."""
                ps_x = psx_pool.tile([128, GRP * 128], bf16, tag="psx")
                for sl in range(GRP):
                    s = GRP * g + sl
                    c, tf = s // CH, s % CH
                    nc.tensor.matmul(
                        ps_x[:, 128 * sl : 128 * (sl + 1)],
                        xi_tiles[c][:, H * tf : H * (tf + 1)],
                        ident_b,
                        is_transpose=True,
                        start=True,
                        stop=True,
                        skip_group_check=True,
                    )
                x4 = x4_pool.tile([128, GRP * 128], bf16, tag="x4")
                nc.scalar.activation(x4, ps_x, AF.Copy)
                return x4

            # init step s=0: p0 = x_0 * exp(start) (fwd), x_511 * exp(end)
            x4_tiles = [xpose_group(0), xpose_group(1)]
            p_state = pst_pool.tile([H, B], bf16, tag="pst")
            nc.scalar.activation(
                p_state[0:BO, :], x4_tiles[0][0:BO, 0:128], AF.Copy,
                scale=exp_st[:, 0:1],
            )
            nc.scalar.activation(
                p_state[BO:H, :], x4_tiles[0][BO:H, 0:128], AF.Copy,
                scale=exp_en[:, 0:1],
            )

            # main scan: s = 1..255, with numerator work sprinkled in.
            # Transpose groups are emitted AFTER the chain ops so the W
            # matmul wins PE priority ties; x4/psx buf rotation gives the
            # lookahead window.
            for s in range(1, NS):
                x4_cur = x4_tiles[s // GRP]
                ps_s = pss_pool.tile([H, B], f32, tag="pss")
                nc.tensor.matmul(ps_s, w_sb, p_state, start=True, stop=True)
                p_new = pst_pool.tile([H, B], bf16, tag="pst")
                nc.vector.tensor_mul(
                    p_new, ps_s, x4_cur[:, 128 * (s % GRP) : 128 * (s % GRP + 1)]
                )
                p_state = p_new
                if s % GRP == 1 and s // GRP + 2 < NS // GRP:
                    x4_tiles.append(xpose_group(s // GRP + 2))
                # numerator DVE/ACT pieces, paced into the second half of
                # the scan (their gpsimd producers are all done by then)
                if s >= 112 and s < 112 + 4 * 32 and (s - 112) % 4 == 0:
                    m = (s - 112) // 4
                    if m % 2 == 0:
                        em_gold_piece(m // 2)
                    else:
                        gather_piece(m // 2)

            # ---------------- final gold assembly ----------------
            em_gold = small_pool.tile([B, 1], f32)
            nc.vector.tensor_reduce(
                out=em_gold, in_=reds_em, axis=mybir.AxisListType.X, op=ALU.add
            )
            nc.sync.dma_start(out=gold_d[:, 0:1], in_=em_gold)
            rest_col = small_pool.tile([B, 1], f32)
            nc.sync.dma_start(out=rest_col, in_=reds_tr[0::16, :])
            nc.sync.dma_start(out=gold_d[:, 1:2], in_=rest_col)

            # ---------------- join ----------------
            ps_j = pss_pool.tile([H, B], f32, tag="pss")
            nc.tensor.matmul(ps_j, w_sb, p_state, start=True, stop=True)
            r_sb = small_pool.tile([NT, B], f32)
            nc.scalar.activation(r_sb, p_state[BO : BO + NT, :], AF.Copy)
            jprod = small_pool.tile([NT, B], f32)
            nc.vector.tensor_mul(jprod, ps_j[0:NT, :], r_sb)
            ps_z = psj_pool.tile([1, B], f32)
            nc.tensor.matmul(ps_z, ones48, jprod, start=True, stop=True)
            zlog_sb = small_pool.tile([1, B], f32)
            nc.scalar.activation(zlog_sb, ps_z, AF.Ln)
            nc.sync.dma_start(out=zlog_d, in_=zlog_sb)

    nc.compile()
    return nc


def _get_nc():
    if "nc" not in _CACHE:
        _CACHE["nc"] = _build()
    return _CACHE["nc"]


def make_indices(tg):
    """Host-side tag bookkeeping: scatter + wrapped-gather index layouts."""
    Bc = tg.shape[0]
    t_ar = np.arange(S)
    sidx = (NT * (t_ar % SC)[None, :] + tg).astype(np.int16)

    gidx = np.full((Bc, NGI), TBL - 1, dtype=np.int16)
    gidx[:, 0 : S - 1] = NT * tg[:, :-1] + tg[:, 1:]
    gidx[:, S - 1] = NT * NT + tg[:, 0]
    gidx[:, S] = NT * NT + NT + tg[:, -1]
    # wrap: gidxw[16g+r, IPG*i+s] = gidx[16g+i, 16s+r]
    g4 = gidx.reshape(Bc // 16, 16, NGI // 16, 16)  # (g, i, s, r)
    gidxw = np.ascontiguousarray(
        g4.transpose(0, 3, 1, 2).reshape(Bc, NGI)
    )  # (16g+r, IPG*i+s)
    return sidx, gidxw


def make_in_maps(emissions, tags, transitions, start_transitions, end_transitions):
    import ml_dtypes

    bf16 = ml_dtypes.bfloat16
    em = np.asarray(emissions, dtype=np.float32).reshape(NCORES * B, S, NT)
    tg = np.ascontiguousarray(np.asarray(tags).astype(np.int64))
    tr = np.ascontiguousarray(np.asarray(transitions, dtype=np.float32))
    st = np.ascontiguousarray(np.asarray(start_transitions, dtype=np.float32))
    en = np.ascontiguousarray(np.asarray(end_transitions, dtype=np.float32))

    # interleaved scan layout: [fwd em_s | pad16 | bwd em_{511-s} | pad16]
    emi = np.zeros((NCORES * B, NS, H), dtype=bf16)
    emi[:, :, 0:NT] = em[:, :NS, :]
    emi[:, :, BO : BO + NT] = em[:, : NS - 1 : -1, :]
    emi = emi.reshape(NCORES * B, NS * H)
    emc = np.ascontiguousarray(em.reshape(NCORES * B, S * NT).astype(bf16))

    in_maps = []
    for c in range(NCORES):
        sl = slice(c * B, (c + 1) * B)
        sidx, gidxw = make_indices(tg[sl])
        in_maps.append(
            {
                "emi": np.ascontiguousarray(emi[sl]),
                "emc": emc[sl],
                "sidx": sidx,
                "gidxw": gidxw,
                "trans": tr,
                "start_t": st,
                "end_t": en,
            }
        )
    return in_maps


def kernel(emissions, tags, mask, transitions, start_transitions, end_transitions):
    from concourse.bass_utils import run_bass_kernel_spmd

    nc = _get_nc()
    in_maps = make_in_maps(
        emissions, tags, transitions, start_transitions, end_transitions
    )
    res = run_bass_kernel_spmd(nc, in_maps, core_ids=list(range(NCORES)))

    total = 0.0
    for r in res.results:
        logz = r["zlog"].astype(np.float64)[0] + (S - 1) * LOG_SCALE
        gold = r["gold"].astype(np.float64)
        total += (logz - gold[:, 0] - gold[:, 1]).sum()
    loss = total / (NCORES * B)
    return np.asarray(loss, dtype=np.float32)


# revision 16
# speedup vs baseline: 1.0832x; 1.0832x over previous
"""CRF negative-log-likelihood kernel for 8 Trainium2 NeuronCores.

Strategy (data-parallel over batch, 128 sequences per core):

Denominator (log-partition) — scaled-probability-space scan in bf16:
    E = exp(T - 7*ln2) in bf16.  Forward chain (t = 0..255) and backward
    chain (t = 511..256) run simultaneously, stacked on partitions 0-47
    and 64-111 of a 128-row state (rows 48-63 / 112-127 carry junk that
    the zero rows of the 128x128 block-diagonal W annihilate each step).
    The host pre-interleaves emissions into 128-wide scan steps
    [fwd em_s | pad | bwd em_{511-s} | pad] in bf16, so that after a
    chunked ACT exp, ONE 128x128 bf16 PE transpose per step produces the
    x-tile in exactly the state layout.  Transposes land in 4-step PSUM
    groups and one ACT copy per group moves them to SBUF (DVE cannot
    read two PSUM operands).  Per scan step: one 128x128 bf16 matmul +
    one DVE multiply.  Join at t=256: Z = sum_i (E^T P_255)[i]*R_256[i];
    logZ = ln(Z) + 511*7*ln2 (host adds the constant).

Numerator (gold-path score):
    em-gold: one-hot built with gpsimd.local_scatter on a compact bf16
    copy of the emissions (scatter 1.0 at 48*(t%32) + tag), DVE bf16
    multiply, ACT-accum reduce per 32-step chunk, final DVE reduce.
    transition/start/end-gold: gather from a replicated 2401-entry table
    [T.flat | start | end | 0] with gpsimd.ap_gather (34-int16-wide
    4-byte-aligned index slices per instruction), ACT-accum reduce.

Outputs per core: zlog (1,128) = ln(Z_scaled) and gold (128,2) =
[em_gold, transition+start+end gold].  Host: loss = mean(zlog +
511*7*ln2 - gold0 - gold1).
"""

import math

import numpy as np

B = 128  # batch rows per core
S = 512
NT = 48
BO = 64  # partition offset of the backward chain
H = 128  # state height
NS = S // 2  # 256 scan steps (incl. init)
NCORES = 8
LOG_SCALE = 7 * math.log(2.0)
CH = 64  # interleaved-chunk size (scan steps)
NCH = NS // CH  # 4 interleaved chunks
CC = 128  # compact-chunk size (time steps)
SC = 32  # scatter chunk size (time steps)
GRP = 4  # scan steps per PSUM transpose group
TBL = NT * NT + NT + NT + 1  # 2401-entry gather table
IPG = 34  # gather idxs per partition per instruction (4-byte aligned)
NGI = 16 * IPG  # 544 gather slots per sequence (511 + 2 + 31 pad)

_CACHE = {}


def _build():
    import concourse.bass as bass
    import concourse.bacc as bacc
    import concourse.tile as tile
    from concourse import mybir
    from concourse.masks import make_identity
    from concourse import library_config
    from concourse.tile import add_dep_helper

    f32 = mybir.dt.float32
    bf16 = mybir.dt.bfloat16
    i16 = mybir.dt.int16
    AF = mybir.ActivationFunctionType
    ALU = mybir.AluOpType

    nc = bacc.Bacc("TRN2", target_bir_lowering=False, debug=False)

    emi_d = nc.dram_tensor("emi", (B, NS * H), bf16, kind="ExternalInput").ap()
    emc_d = nc.dram_tensor("emc", (B, S * NT), bf16, kind="ExternalInput").ap()
    sidx_d = nc.dram_tensor("sidx", (B, S), i16, kind="ExternalInput").ap()
    gidx_d = nc.dram_tensor("gidxw", (B, NGI), i16, kind="ExternalInput").ap()
    trans_d = nc.dram_tensor("trans", (NT, NT), f32, kind="ExternalInput").ap()
    start_d = nc.dram_tensor("start_t", (NT,), f32, kind="ExternalInput").ap()
    end_d = nc.dram_tensor("end_t", (NT,), f32, kind="ExternalInput").ap()
    zlog_d = nc.dram_tensor("zlog", (1, B), f32, kind="ExternalOutput").ap()
    gold_d = nc.dram_tensor("gold", (B, 2), f32, kind="ExternalOutput").ap()

    with tile.TileContext(nc) as tc:
        with (
            tc.tile_pool(name="consts", bufs=1) as consts,
            tc.tile_pool(name="emi", bufs=2) as emi_pool,
            tc.tile_pool(name="xi", bufs=2) as xi_pool,
            tc.tile_pool(name="emc", bufs=2) as emc_pool,
            tc.tile_pool(name="x4", bufs=3) as x4_pool,
            tc.tile_pool(name="pst", bufs=3) as pst_pool,
            tc.tile_pool(name="oh", bufs=16) as oh_pool,
            tc.tile_pool(name="num", bufs=2) as num_pool,
            tc.tile_pool(name="gout", bufs=16) as gout_pool,
            tc.tile_pool(name="scr", bufs=1) as scr_pool,
            tc.tile_pool(name="small", bufs=2) as small_pool,
            tc.tile_pool(name="psx", bufs=3, space="PSUM") as psx_pool,
            tc.tile_pool(name="pss", bufs=2, space="PSUM") as pss_pool,
            tc.tile_pool(name="psj", bufs=1, space="PSUM") as psj_pool,
        ):
            # ---------------- constants ----------------
            ident_b = consts.tile([128, 128], bf16)
            make_identity(nc, ident_b)
            ident_f = consts.tile([NT, NT], f32)
            make_identity(nc, ident_f)

            t_sb = consts.tile([NT, NT], f32)
            nc.sync.dma_start(out=t_sb, in_=trans_d)
            # start/end padded to 64 rows (rows 48:64 = 0 -> exp = 1) so
            # the init ACT copies cover the junk rows with finite data.
            start_sb = consts.tile([BO, 1], f32)
            nc.vector.memset(start_sb, 0.0)
            nc.sync.dma_start(out=start_sb[0:NT, :], in_=start_d)
            end_sb = consts.tile([BO, 1], f32)
            nc.vector.memset(end_sb, 0.0)
            nc.sync.dma_start(out=end_sb[0:NT, :], in_=end_d)

            exp_st = consts.tile([BO, 1], f32)
            nc.scalar.activation(exp_st, start_sb, AF.Exp)
            exp_en = consts.tile([BO, 1], f32)
            nc.scalar.activation(exp_en, end_sb, AF.Exp)

            ones48 = consts.tile([NT, 1], f32)
            nc.vector.memset(ones48, 1.0)
            nls = consts.tile([NT, 1], f32)
            nc.vector.memset(nls, -LOG_SCALE)

            # W = blockdiag(E @0, 0, E^T @64), E = exp(T - LOG_SCALE), bf16
            w_sb = consts.tile([H, H], bf16)
            nc.vector.memset(w_sb, 0.0)
            ps_tt = psj_pool.tile([NT, NT], f32)
            nc.tensor.transpose(ps_tt, t_sb, ident_f)
            nc.scalar.activation(w_sb[0:NT, 0:NT], t_sb, AF.Exp, bias=nls[:, 0:1])
            nc.scalar.activation(
                w_sb[BO : BO + NT, BO : BO + NT], ps_tt, AF.Exp, bias=nls[:, 0:1]
            )

            # gather table [T.flat | start | end | 0] replicated on 128 parts
            table = consts.tile([B, TBL], f32)
            nc.sync.dma_start(
                out=table[:, 0 : NT * NT],
                in_=bass.AP(
                    tensor=trans_d.tensor,
                    offset=trans_d.offset,
                    ap=[[0, B], [1, NT * NT]],
                ),
            )
            nc.sync.dma_start(
                out=table[:, NT * NT : NT * NT + NT],
                in_=bass.AP(
                    tensor=start_d.tensor,
                    offset=start_d.offset,
                    ap=[[0, B], [1, NT]],
                ),
            )
            nc.sync.dma_start(
                out=table[:, NT * NT + NT : NT * NT + 2 * NT],
                in_=bass.AP(
                    tensor=end_d.tensor,
                    offset=end_d.offset,
                    ap=[[0, B], [1, NT]],
                ),
            )
            nc.vector.memset(table[:, TBL - 1 : TBL], 0.0)

            data_ones = consts.tile([B, SC], bf16)
            nc.vector.memset(data_ones, 1.0)

            idx16 = consts.tile([B, S], i16)
            nc.sync.dma_start(out=idx16, in_=sidx_d)
            gidx16 = consts.tile([B, NGI], i16)
            nc.sync.dma_start(out=gidx16, in_=gidx_d)

            # ---------------- chunked loads + exp ----------------
            xi_tiles = {}
            for c in range(NCH):
                emt = emi_pool.tile([B, CH * H], bf16, tag="emi")
                nc.sync.dma_start(
                    out=emt, in_=emi_d[:, H * CH * c : H * CH * (c + 1)]
                )
                xt = xi_pool.tile([B, CH * H], bf16, tag="xi")
                nc.scalar.activation(xt, emt, AF.Exp)
                xi_tiles[c] = xt

            emc_tiles = {}
            for c in range(S // CC):
                emt = emc_pool.tile([B, CC * NT], bf16, tag="emc")
                nc.sync.dma_start(
                    out=emt, in_=emc_d[:, NT * CC * c : NT * CC * (c + 1)]
                )
                emc_tiles[c] = emt

            # ---------------- numerator: scatters (gpsimd, early) --------
            ld_ls = nc.gpsimd.load_library(library_config.local_scatter)
            scatter_insts = []
            oh_tiles = []
            for k in range(S // SC):
                oh = oh_pool.tile([B, SC * NT], bf16, tag="oh")
                sc_i = nc.gpsimd.local_scatter(
                    out_ap=oh,
                    data_ap=data_ones,
                    idxs_ap=idx16[:, SC * k : SC * (k + 1)],
                    channels=B,
                    num_elems=SC * NT,
                    num_idxs=SC,
                )
                add_dep_helper(sc_i.ins, ld_ls.ins, reason="lib order")
                scatter_insts.append(sc_i)
                oh_tiles.append(oh)

            ld_ag = nc.gpsimd.load_library(library_config.ap_gather)
            for sc_i in scatter_insts:
                add_dep_helper(ld_ag.ins, sc_i.ins, reason="lib order")

            reds_em = consts.tile([B, S // SC], f32)
            reds_tr = consts.tile([B, 16], f32)

            def em_gold_piece(k):
                """DVE mul + ACT accum-reduce for scatter chunk k."""
                c, hh = k // 4, k % 4
                prod = num_pool.tile([B, SC * NT], bf16, tag="prod")
                nc.vector.tensor_mul(
                    prod,
                    emc_tiles[c][:, SC * NT * hh : SC * NT * (hh + 1)],
                    oh_tiles[k],
                )
                scr = scr_pool.tile([B, SC * NT], bf16, tag="scr")
                nc.scalar.activation(
                    scr, prod, AF.Copy, accum_out=reds_em[:, k : k + 1]
                )

            # all 16 gathers issued up front so the gpsimd queue runs
            # [scatters, lib, gathers] before any tile-release bookkeeping
            g_out_tiles = []
            for i in range(16):
                g_out = gout_pool.tile([B, NGI], f32, tag="gout")
                ag_i = nc.gpsimd.ap_gather(
                    out_ap=g_out,
                    in_ap=table,
                    idxs_ap=gidx16[:, IPG * i : IPG * (i + 1)],
                    channels=B,
                    num_elems=TBL,
                    d=1,
                    num_idxs=NGI,
                )
                add_dep_helper(ag_i.ins, ld_ag.ins, reason="lib order")
                g_out_tiles.append(g_out)

            def gather_piece(i):
                nc.vector.tensor_reduce(
                    out=reds_tr[:, i : i + 1],
                    in_=g_out_tiles[i],
                    axis=mybir.AxisListType.X,
                    op=ALU.add,
                )

            # ---------------- scan ----------------
            def xpose_group(g):
                """Transpose x for scan steps 4g..4g+3 into one PSUM tile,
                then one ACT copy to SBUF bf16."""
                ps_x = psx_pool.tile([128, GRP * 128], bf16, tag="psx")
                for sl in range(GRP):
                    s = GRP * g + sl
                    c, tf = s // CH, s % CH
                    nc.tensor.matmul(
                        ps_x[:, 128 * sl : 128 * (sl + 1)],
                        xi_tiles[c][:, H * tf : H * (tf + 1)],
                        ident_b,
                        is_transpose=True,
                        start=True,
                        stop=True,
                        skip_group_check=True,
                    )
                x4 = x4_pool.tile([128, GRP * 128], bf16, tag="x4")
                nc.scalar.activation(x4, ps_x, AF.Copy)
                return x4

            # init step s=0: p0 = x_0 * exp(start) (fwd), x_511 * exp(end)
            x4_tiles = [xpose_group(0), xpose_group(1)]
            p_state = pst_pool.tile([H, B], bf16, tag="pst")
            nc.scalar.activation(
                p_state[0:BO, :], x4_tiles[0][0:BO, 0:128], AF.Copy,
                scale=exp_st[:, 0:1],
            )
            nc.scalar.activation(
                p_state[BO:H, :], x4_tiles[0][BO:H, 0:128], AF.Copy,
                scale=exp_en[:, 0:1],
            )

            # main scan: s = 1..255, with numerator work sprinkled in.
            # Transpose groups are emitted AFTER the chain ops so the W
            # matmul wins PE priority ties; x4/psx buf rotation gives the
            # lookahead window.
            for s in range(1, NS):
                x4_cur = x4_tiles[s // GRP]
                ps_s = pss_pool.tile([H, B], f32, tag="pss")
                nc.tensor.matmul(ps_s, w_sb, p_state, start=True, stop=True)
                p_new = pst_pool.tile([H, B], bf16, tag="pst")
                nc.vector.tensor_mul(
                    p_new, ps_s, x4_cur[:, 128 * (s % GRP) : 128 * (s % GRP + 1)]
                )
                p_state = p_new
                if s % GRP == 1 and s // GRP + 2 < NS // GRP:
                    x4_tiles.append(xpose_group(s // GRP + 2))
                # numerator DVE/ACT pieces, paced into the second half of
                # the scan (their gpsimd producers are all done by then)
                if 112 <= s < 128:
                    gather_piece(s - 112)
                elif 130 <= s < 162 and s % 2 == 0:
                    em_gold_piece((s - 130) // 2)

            # ---------------- final gold assembly ----------------
            em_gold = small_pool.tile([B, 1], f32)
            nc.vector.tensor_reduce(
                out=em_gold, in_=reds_em, axis=mybir.AxisListType.X, op=ALU.add
            )
            nc.sync.dma_start(out=gold_d[:, 0:1], in_=em_gold)
            rest_col = small_pool.tile([B, 1], f32)
            nc.sync.dma_start(out=rest_col, in_=reds_tr[0::16, :])
            nc.sync.dma_start(out=gold_d[:, 1:2], in_=rest_col)

            # ---------------- join ----------------
            ps_j = pss_pool.tile([H, B], f32, tag="pss")
            nc.tensor.matmul(ps_j, w_sb, p_state, start=True, stop=True)
            r_sb = small_pool.tile([NT, B], f32)
            nc.scalar.activation(r_sb, p_state[BO : BO + NT, :], AF.Copy)
            jprod = small_pool.tile([NT, B], f32)
            nc.vector.tensor_mul(jprod, ps_j[0:NT, :], r_sb)
            ps_z = psj_pool.tile([1, B], f32)
            nc.tensor.matmul(ps_z, ones48, jprod, start=True, stop=True)
            zlog_sb = small_pool.tile([1, B], f32)
            nc.scalar.activation(zlog_sb, ps_z, AF.Ln)
            nc.sync.dma_start(out=zlog_d, in_=zlog_sb)

    nc.compile()
    return nc


def _get_nc():
    if "nc" not in _CACHE:
        _CACHE["nc"] = _build()
    return _CACHE["nc"]


def make_indices(tg):
    """Host-side tag bookkeeping: scatter + wrapped-gather index layouts."""
    Bc = tg.shape[0]
    t_ar = np.arange(S)
    sidx = (NT * (t_ar % SC)[None, :] + tg).astype(np.int16)

    gidx = np.full((Bc, NGI), TBL - 1, dtype=np.int16)
    gidx[:, 0 : S - 1] = NT * tg[:, :-1] + tg[:, 1:]
    gidx[:, S - 1] = NT * NT + tg[:, 0]
    gidx[:, S] = NT * NT + NT + tg[:, -1]
    # wrap: gidxw[16g+r, IPG*i+s] = gidx[16g+i, 16s+r]
    g4 = gidx.reshape(Bc // 16, 16, NGI // 16, 16)  # (g, i, s, r)
    gidxw = np.ascontiguousarray(
        g4.transpose(0, 3, 1, 2).reshape(Bc, NGI)
    )  # (16g+r, IPG*i+s)
    return sidx, gidxw


def make_in_maps(emissions, tags, transitions, start_transitions, end_transitions):
    import ml_dtypes

    bf16 = ml_dtypes.bfloat16
    em = np.asarray(emissions, dtype=np.float32).reshape(NCORES * B, S, NT)
    tg = np.ascontiguousarray(np.asarray(tags).astype(np.int64))
    tr = np.ascontiguousarray(np.asarray(transitions, dtype=np.float32))
    st = np.ascontiguousarray(np.asarray(start_transitions, dtype=np.float32))
    en = np.ascontiguousarray(np.asarray(end_transitions, dtype=np.float32))

    # interleaved scan layout: [fwd em_s | pad16 | bwd em_{511-s} | pad16]
    emi = np.zeros((NCORES * B, NS, H), dtype=bf16)
    emi[:, :, 0:NT] = em[:, :NS, :]
    emi[:, :, BO : BO + NT] = em[:, : NS - 1 : -1, :]
    emi = emi.reshape(NCORES * B, NS * H)
    emc = np.ascontiguousarray(em.reshape(NCORES * B, S * NT).astype(bf16))

    in_maps = []
    for c in range(NCORES):
        sl = slice(c * B, (c + 1) * B)
        sidx, gidxw = make_indices(tg[sl])
        in_maps.append(
            {
                "emi": np.ascontiguousarray(emi[sl]),
                "emc": emc[sl],
                "sidx": sidx,
                "gidxw": gidxw,
                "trans": tr,
                "start_t": st,
                "end_t": en,
            }
        )
    return in_maps


def kernel(emissions, tags, mask, transitions, start_transitions, end_transitions):
    from concourse.bass_utils import run_bass_kernel_spmd

    nc = _get_nc()
    in_maps = make_in_maps(
        emissions, tags, transitions, start_transitions, end_transitions
    )
    res = run_bass_kernel_spmd(nc, in_maps, core_ids=list(range(NCORES)))

    total = 0.0
    for r in res.results:
        logz = r["zlog"].astype(np.float64)[0] + (S - 1) * LOG_SCALE
        gold = r["gold"].astype(np.float64)
        total += (logz - gold[:, 0] - gold[:, 1]).sum()
    loss = total / (NCORES * B)
    return np.asarray(loss, dtype=np.float32)


# revision 18
# speedup vs baseline: 2.0824x; 1.9224x over previous
"""CRF negative-log-likelihood kernel for 8 Trainium2 NeuronCores.

Strategy (data-parallel over batch, 128 sequences per core):

Denominator (log-partition) — scaled-probability-space scan in bf16:
    E = exp(T - 7*ln2) in bf16.  Forward chain (t = 0..255) and backward
    chain (t = 511..256) run simultaneously, stacked on partitions 0-47
    and 64-111 of a 128-row state (rows 48-63 / 112-127 carry junk that
    the zero rows of the 128x128 block-diagonal W annihilate each step).
    The host pre-interleaves emissions into 128-wide scan steps
    [fwd em_s | pad | bwd em_{511-s} | pad] in bf16, so that after a
    chunked ACT exp, ONE 128x128 bf16 PE transpose per step produces the
    x-tile in exactly the state layout.  Transposes land in 4-step PSUM
    groups and one ACT copy per group moves them to SBUF (DVE cannot
    read two PSUM operands).  Per scan step: one 128x128 bf16 matmul +
    one DVE multiply.  Join at t=256: Z = sum_i (E^T P_255)[i]*R_256[i];
    logZ = ln(Z) + 511*7*ln2 (host adds the constant).

Numerator (gold-path score):
    em-gold: one-hot built with gpsimd.local_scatter on a compact bf16
    copy of the emissions (scatter 1.0 at 48*(t%32) + tag), DVE bf16
    multiply, ACT-accum reduce per 32-step chunk, final DVE reduce.
    transition/start/end-gold: gather from a replicated 2401-entry table
    [T.flat | start | end | 0] with gpsimd.ap_gather (34-int16-wide
    4-byte-aligned index slices per instruction), ACT-accum reduce.

Outputs per core: zlog (1,128) = ln(Z_scaled) and gold (128,2) =
[em_gold, transition+start+end gold].  Host: loss = mean(zlog +
511*7*ln2 - gold0 - gold1).
"""

import math

import numpy as np

B = 128  # batch rows per core
S = 512
NT = 48
BO = 64  # partition offset of the backward chain
H = 128  # state height
NS = S // 2  # 256 scan steps (incl. init)
NCORES = 8
LOG_SCALE = 7 * math.log(2.0)
CH = 64  # interleaved-chunk size (scan steps)
NCH = NS // CH  # 4 interleaved chunks
CC = 128  # compact-chunk size (time steps)
SC = 32  # scatter chunk size (time steps)
GRP = 4  # scan steps per PSUM transpose group
KCT = 2432  # count-vector length: 2304 pairs + 48 start + 48 end + 32 pad
NKC = KCT // 128  # 19 contraction chunks

_CACHE = {}


def _build():
    import concourse.bass as bass
    import concourse.bacc as bacc
    import concourse.tile as tile
    from concourse import mybir
    from concourse.masks import make_identity
    from concourse import library_config
    from concourse.tile import add_dep_helper

    f32 = mybir.dt.float32
    bf16 = mybir.dt.bfloat16
    i16 = mybir.dt.int16
    AF = mybir.ActivationFunctionType
    ALU = mybir.AluOpType

    nc = bacc.Bacc("TRN2", target_bir_lowering=False, debug=False)

    emi_d = nc.dram_tensor("emi", (B, NS * H), bf16, kind="ExternalInput").ap()
    emc_d = nc.dram_tensor("emc", (B, S * NT), bf16, kind="ExternalInput").ap()
    sidx_d = nc.dram_tensor("sidx", (B, S), i16, kind="ExternalInput").ap()
    ct_d = nc.dram_tensor("ct", (KCT, B), bf16, kind="ExternalInput").ap()
    trans_d = nc.dram_tensor("trans", (NT, NT), f32, kind="ExternalInput").ap()
    start_d = nc.dram_tensor("start_t", (NT,), f32, kind="ExternalInput").ap()
    end_d = nc.dram_tensor("end_t", (NT,), f32, kind="ExternalInput").ap()
    zlog_d = nc.dram_tensor("zlog", (1, B), f32, kind="ExternalOutput").ap()
    gold_d = nc.dram_tensor("gold", (B, 2), f32, kind="ExternalOutput").ap()

    with tile.TileContext(nc) as tc:
        with (
            tc.tile_pool(name="consts", bufs=1) as consts,
            tc.tile_pool(name="emi", bufs=2) as emi_pool,
            tc.tile_pool(name="xi", bufs=2) as xi_pool,
            tc.tile_pool(name="emc", bufs=2) as emc_pool,
            tc.tile_pool(name="x4", bufs=3) as x4_pool,
            tc.tile_pool(name="pst", bufs=3) as pst_pool,
            tc.tile_pool(name="oh", bufs=16) as oh_pool,
            tc.tile_pool(name="num", bufs=2) as num_pool,
            tc.tile_pool(name="scr", bufs=1) as scr_pool,
            tc.tile_pool(name="small", bufs=2) as small_pool,
            tc.tile_pool(name="psx", bufs=3, space="PSUM") as psx_pool,
            tc.tile_pool(name="pss", bufs=2, space="PSUM") as pss_pool,
            tc.tile_pool(name="psj", bufs=1, space="PSUM") as psj_pool,
        ):
            # ---------------- constants ----------------
            ident_b = consts.tile([128, 128], bf16)
            make_identity(nc, ident_b)
            ident_f = consts.tile([NT, NT], f32)
            make_identity(nc, ident_f)

            t_sb = consts.tile([NT, NT], f32)
            nc.sync.dma_start(out=t_sb, in_=trans_d)
            # start/end padded to 64 rows (rows 48:64 = 0 -> exp = 1) so
            # the init ACT copies cover the junk rows with finite data.
            start_sb = consts.tile([BO, 1], f32)
            nc.vector.memset(start_sb, 0.0)
            nc.sync.dma_start(out=start_sb[0:NT, :], in_=start_d)
            end_sb = consts.tile([BO, 1], f32)
            nc.vector.memset(end_sb, 0.0)
            nc.sync.dma_start(out=end_sb[0:NT, :], in_=end_d)

            exp_st = consts.tile([BO, 1], f32)
            nc.scalar.activation(exp_st, start_sb, AF.Exp)
            exp_en = consts.tile([BO, 1], f32)
            nc.scalar.activation(exp_en, end_sb, AF.Exp)

            ones48 = consts.tile([NT, 1], f32)
            nc.vector.memset(ones48, 1.0)
            nls = consts.tile([NT, 1], f32)
            nc.vector.memset(nls, -LOG_SCALE)

            # W = blockdiag(E @0, 0, E^T @64), E = exp(T - LOG_SCALE), bf16
            w_sb = consts.tile([H, H], bf16)
            nc.vector.memset(w_sb, 0.0)
            ps_tt = psj_pool.tile([NT, NT], f32)
            nc.tensor.transpose(ps_tt, t_sb, ident_f)
            nc.scalar.activation(w_sb[0:NT, 0:NT], t_sb, AF.Exp, bias=nls[:, 0:1])
            nc.scalar.activation(
                w_sb[BO : BO + NT, BO : BO + NT], ps_tt, AF.Exp, bias=nls[:, 0:1]
            )

            data_ones = consts.tile([B, SC], bf16)
            nc.vector.memset(data_ones, 1.0)

            idx16 = consts.tile([B, S], i16)
            nc.sync.dma_start(out=idx16, in_=sidx_d)

            # pair-count matrix chunks (contraction rows on partitions) and
            # the flat [T | start | end | pad] value vector, column per chunk
            ct_sb = consts.tile([128, KCT], bf16)
            for k in range(NKC):
                nc.sync.dma_start(
                    out=ct_sb[:, 128 * k : 128 * (k + 1)],
                    in_=ct_d[128 * k : 128 * (k + 1), :],
                )
            tf_f = consts.tile([128, NKC], f32)
            nc.sync.dma_start(
                out=tf_f[:, 0 : NKC - 1],
                in_=bass.AP(
                    tensor=trans_d.tensor,
                    offset=trans_d.offset,
                    ap=[[1, 128], [128, NKC - 1]],
                ),
            )
            nc.vector.memset(tf_f[:, NKC - 1 : NKC], 0.0)
            nc.sync.dma_start(
                out=tf_f[0:NT, NKC - 1 : NKC],
                in_=bass.AP(
                    tensor=start_d.tensor, offset=start_d.offset, ap=[[1, NT]]
                ),
            )
            nc.sync.dma_start(
                out=tf_f[NT : 2 * NT, NKC - 1 : NKC],
                in_=bass.AP(
                    tensor=end_d.tensor, offset=end_d.offset, ap=[[1, NT]]
                ),
            )
            tf_sb = consts.tile([128, NKC], bf16)
            nc.scalar.activation(tf_sb, tf_f, AF.Copy)

            # ---------------- chunked loads + exp ----------------
            xi_tiles = {}
            for c in range(NCH):
                emt = emi_pool.tile([B, CH * H], bf16, tag="emi")
                nc.sync.dma_start(
                    out=emt, in_=emi_d[:, H * CH * c : H * CH * (c + 1)]
                )
                xt = xi_pool.tile([B, CH * H], bf16, tag="xi")
                nc.scalar.activation(xt, emt, AF.Exp)
                xi_tiles[c] = xt

            emc_tiles = {}
            for c in range(S // CC):
                emt = emc_pool.tile([B, CC * NT], bf16, tag="emc")
                nc.sync.dma_start(
                    out=emt, in_=emc_d[:, NT * CC * c : NT * CC * (c + 1)]
                )
                emc_tiles[c] = emt

            # ---------------- numerator: scatters (gpsimd, early) --------
            ld_ls = nc.gpsimd.load_library(library_config.local_scatter)
            scatter_insts = []
            oh_tiles = []
            for k in range(S // SC):
                oh = oh_pool.tile([B, SC * NT], bf16, tag="oh")
                sc_i = nc.gpsimd.local_scatter(
                    out_ap=oh,
                    data_ap=data_ones,
                    idxs_ap=idx16[:, SC * k : SC * (k + 1)],
                    channels=B,
                    num_elems=SC * NT,
                    num_idxs=SC,
                )
                add_dep_helper(sc_i.ins, ld_ls.ins, reason="lib order")
                scatter_insts.append(sc_i)
                oh_tiles.append(oh)

            reds_em = consts.tile([B, S // SC], f32)

            def em_gold_piece(k):
                """DVE mul + ACT accum-reduce for scatter chunk k."""
                c, hh = k // 4, k % 4
                prod = num_pool.tile([B, SC * NT], bf16, tag="prod")
                nc.vector.tensor_mul(
                    prod,
                    emc_tiles[c][:, SC * NT * hh : SC * NT * (hh + 1)],
                    oh_tiles[k],
                )
                scr = scr_pool.tile([B, SC * NT], bf16, tag="scr")
                nc.scalar.activation(
                    scr, prod, AF.Copy, accum_out=reds_em[:, k : k + 1]
                )

            # ---------------- scan ----------------
            def xpose_group(g):
                """Transpose x for scan steps 4g..4g+3 into one PSUM tile,
                then one ACT copy to SBUF bf16."""
                ps_x = psx_pool.tile([128, GRP * 128], bf16, tag="psx")
                for sl in range(GRP):
                    s = GRP * g + sl
                    c, tf = s // CH, s % CH
                    nc.tensor.matmul(
                        ps_x[:, 128 * sl : 128 * (sl + 1)],
                        xi_tiles[c][:, H * tf : H * (tf + 1)],
                        ident_b,
                        is_transpose=True,
                        start=True,
                        stop=True,
                        skip_group_check=True,
                    )
                x4 = x4_pool.tile([128, GRP * 128], bf16, tag="x4")
                nc.scalar.activation(x4, ps_x, AF.Copy)
                return x4

            # init step s=0: p0 = x_0 * exp(start) (fwd), x_511 * exp(end)
            x4_tiles = [xpose_group(0), xpose_group(1)]
            p_state = pst_pool.tile([H, B], bf16, tag="pst")
            nc.scalar.activation(
                p_state[0:BO, :], x4_tiles[0][0:BO, 0:128], AF.Copy,
                scale=exp_st[:, 0:1],
            )
            nc.scalar.activation(
                p_state[BO:H, :], x4_tiles[0][BO:H, 0:128], AF.Copy,
                scale=exp_en[:, 0:1],
            )

            # main scan: s = 1..255, with numerator work sprinkled in.
            # Transpose groups are emitted AFTER the chain ops so the W
            # matmul wins PE priority ties; x4/psx buf rotation gives the
            # lookahead window.
            for s in range(1, NS):
                x4_cur = x4_tiles[s // GRP]
                ps_s = pss_pool.tile([H, B], f32, tag="pss")
                nc.tensor.matmul(ps_s, w_sb, p_state, start=True, stop=True)
                p_new = pst_pool.tile([H, B], bf16, tag="pst")
                nc.vector.tensor_mul(
                    p_new, ps_s, x4_cur[:, 128 * (s % GRP) : 128 * (s % GRP + 1)]
                )
                p_state = p_new
                if s % GRP == 1 and s // GRP + 2 < NS // GRP:
                    x4_tiles.append(xpose_group(s // GRP + 2))
                # numerator DVE/ACT pieces, paced into the second half of
                # the scan (their gpsimd producers are all done by then)
                if 112 <= s < 112 + 4 * 16 and (s - 112) % 4 == 0:
                    em_gold_piece((s - 112) // 4)

            # ---------------- final gold assembly ----------------
            em_gold = small_pool.tile([B, 1], f32)
            nc.vector.tensor_reduce(
                out=em_gold, in_=reds_em, axis=mybir.AxisListType.X, op=ALU.add
            )
            nc.sync.dma_start(out=gold_d[:, 0:1], in_=em_gold)
            # rest_gold[b] = sum_k C_b[k] * Tflat[k]: 19 accumulating matmuls
            ps_c = psj_pool.tile([B, 1], f32)
            for k in range(NKC):
                nc.tensor.matmul(
                    ps_c,
                    ct_sb[:, 128 * k : 128 * (k + 1)],
                    tf_sb[:, k : k + 1],
                    start=(k == 0),
                    stop=(k == NKC - 1),
                )
            rest_col = small_pool.tile([B, 1], f32)
            nc.scalar.activation(rest_col, ps_c, AF.Copy)
            nc.sync.dma_start(out=gold_d[:, 1:2], in_=rest_col)

            # ---------------- join ----------------
            ps_j = pss_pool.tile([H, B], f32, tag="pss")
            nc.tensor.matmul(ps_j, w_sb, p_state, start=True, stop=True)
            r_sb = small_pool.tile([NT, B], f32)
            nc.scalar.activation(r_sb, p_state[BO : BO + NT, :], AF.Copy)
            jprod = small_pool.tile([NT, B], f32)
            nc.vector.tensor_mul(jprod, ps_j[0:NT, :], r_sb)
            ps_z = psj_pool.tile([1, B], f32)
            nc.tensor.matmul(ps_z, ones48, jprod, start=True, stop=True)
            zlog_sb = small_pool.tile([1, B], f32)
            nc.scalar.activation(zlog_sb, ps_z, AF.Ln)
            nc.sync.dma_start(out=zlog_d, in_=zlog_sb)

    nc.compile()
    return nc


def _get_nc():
    if "nc" not in _CACHE:
        _CACHE["nc"] = _build()
    return _CACHE["nc"]


def make_indices(tg):
    """Host-side tag bookkeeping: scatter indices + pair-count matrix."""
    import ml_dtypes

    Bc = tg.shape[0]
    t_ar = np.arange(S)
    sidx = (NT * (t_ar % SC)[None, :] + tg).astype(np.int16)

    pair = NT * tg[:, :-1] + tg[:, 1:]  # (Bc, S-1)
    bi = np.repeat(np.arange(Bc), S - 1)
    flat = bi * KCT + pair.reshape(-1)
    ct = np.bincount(flat, minlength=Bc * KCT).reshape(Bc, KCT)
    ct[np.arange(Bc), NT * NT + tg[:, 0]] += 1
    ct[np.arange(Bc), NT * NT + NT + tg[:, -1]] += 1
    ct = np.ascontiguousarray(ct.T.astype(ml_dtypes.bfloat16))  # (KCT, Bc)
    return sidx, ct


def make_in_maps(emissions, tags, transitions, start_transitions, end_transitions):
    import ml_dtypes

    bf16 = ml_dtypes.bfloat16
    em = np.asarray(emissions, dtype=np.float32).reshape(NCORES * B, S, NT)
    tg = np.ascontiguousarray(np.asarray(tags).astype(np.int64))
    tr = np.ascontiguousarray(np.asarray(transitions, dtype=np.float32))
    st = np.ascontiguousarray(np.asarray(start_transitions, dtype=np.float32))
    en = np.ascontiguousarray(np.asarray(end_transitions, dtype=np.float32))

    # interleaved scan layout: [fwd em_s | pad16 | bwd em_{511-s} | pad16]
    emi = np.zeros((NCORES * B, NS, H), dtype=bf16)
    emi[:, :, 0:NT] = em[:, :NS, :]
    emi[:, :, BO : BO + NT] = em[:, : NS - 1 : -1, :]
    emi = emi.reshape(NCORES * B, NS * H)
    emc = np.ascontiguousarray(em.reshape(NCORES * B, S * NT).astype(bf16))

    in_maps = []
    for c in range(NCORES):
        sl = slice(c * B, (c + 1) * B)
        sidx, ct = make_indices(tg[sl])
        in_maps.append(
            {
                "emi": np.ascontiguousarray(emi[sl]),
                "emc": emc[sl],
                "sidx": sidx,
                "ct": ct,
                "trans": tr,
                "start_t": st,
                "end_t": en,
            }
        )
    return in_maps


def kernel(emissions, tags, mask, transitions, start_transitions, end_transitions):
    from concourse.bass_utils import run_bass_kernel_spmd

    nc = _get_nc()
    in_maps = make_in_maps(
        emissions, tags, transitions, start_transitions, end_transitions
    )
    res = run_bass_kernel_spmd(nc, in_maps, core_ids=list(range(NCORES)))

    total = 0.0
    for r in res.results:
        logz = r["zlog"].astype(np.float64)[0] + (S - 1) * LOG_SCALE
        gold = r["gold"].astype(np.float64)
        total += (logz - gold[:, 0] - gold[:, 1]).sum()
    loss = total / (NCORES * B)
    return np.asarray(loss, dtype=np.float32)


# revision 20
# speedup vs baseline: 2.1596x; 1.0371x over previous
"""CRF negative-log-likelihood kernel for 8 Trainium2 NeuronCores.

Strategy (data-parallel over batch, 128 sequences per core):

Denominator (log-partition) — scaled-probability-space scan in bf16:
    E = exp(T - 7*ln2) in bf16.  Forward chain (t = 0..255) and backward
    chain (t = 511..256) run simultaneously, stacked on partitions 0-47
    and 64-111 of a 128-row state (rows 48-63 / 112-127 carry junk that
    the zero rows of the 128x128 block-diagonal W annihilate each step).
    The host pre-interleaves emissions into 128-wide scan steps
    [fwd em_s | pad | bwd em_{511-s} | pad] in bf16, so that after a
    chunked ACT exp, ONE 128x128 bf16 PE transpose per step produces the
    x-tile in exactly the state layout.  Transposes land in 4-step PSUM
    groups and one ACT copy per group moves them to SBUF (DVE cannot
    read two PSUM operands).  Per scan step: one 128x128 bf16 matmul +
    one DVE multiply.  Join at t=256: Z = sum_i (E^T P_255)[i]*R_256[i];
    logZ = ln(Z) + 511*7*ln2 (host adds the constant).

Numerator (gold-path score):
    em-gold: one-hot built with gpsimd.local_scatter on a compact bf16
    copy of the emissions (scatter 1.0 at 48*(t%32) + tag), DVE bf16
    multiply, ACT-accum reduce per 32-step chunk, final DVE reduce.
    transition/start/end-gold: gather from a replicated 2401-entry table
    [T.flat | start | end | 0] with gpsimd.ap_gather (34-int16-wide
    4-byte-aligned index slices per instruction), ACT-accum reduce.

Outputs per core: zlog (1,128) = ln(Z_scaled) and gold (128,2) =
[em_gold, transition+start+end gold].  Host: loss = mean(zlog +
511*7*ln2 - gold0 - gold1).
"""

import math

import numpy as np

B = 128  # batch rows per core
S = 512
NT = 48
BO = 64  # partition offset of the backward chain
H = 128  # state height
NS = S // 2  # 256 scan steps (incl. init)
NCORES = 8
LOG_SCALE = 7 * math.log(2.0)
CH = 64  # interleaved-chunk size (scan steps)
NCH = NS // CH  # 4 interleaved chunks
CC = 128  # compact-chunk size (time steps)
SC = 32  # scatter chunk size (time steps)
GRP = 4  # scan steps per PSUM transpose group
KCT = 2432  # count-vector length: 2304 pairs + 48 start + 48 end + 32 pad
NKC = KCT // 128  # 19 contraction chunks

_CACHE = {}


def _build():
    import concourse.bass as bass
    import concourse.bacc as bacc
    import concourse.tile as tile
    from concourse import mybir
    from concourse.masks import make_identity
    from concourse import library_config
    from concourse.tile import add_dep_helper

    f32 = mybir.dt.float32
    bf16 = mybir.dt.bfloat16
    i16 = mybir.dt.int16
    AF = mybir.ActivationFunctionType
    ALU = mybir.AluOpType

    nc = bacc.Bacc("TRN2", target_bir_lowering=False, debug=False)

    emi_d = nc.dram_tensor("emi", (B, NS * H), bf16, kind="ExternalInput").ap()
    emc_d = nc.dram_tensor("emc", (B, S * NT), bf16, kind="ExternalInput").ap()
    sidx_d = nc.dram_tensor("sidx", (B, S), i16, kind="ExternalInput").ap()
    ct_d = nc.dram_tensor("ct", (KCT, B), bf16, kind="ExternalInput").ap()
    trans_d = nc.dram_tensor("trans", (NT, NT), f32, kind="ExternalInput").ap()
    start_d = nc.dram_tensor("start_t", (NT,), f32, kind="ExternalInput").ap()
    end_d = nc.dram_tensor("end_t", (NT,), f32, kind="ExternalInput").ap()
    zlog_d = nc.dram_tensor("zlog", (1, B), f32, kind="ExternalOutput").ap()
    gold_d = nc.dram_tensor("gold", (B, 2), f32, kind="ExternalOutput").ap()

    with tile.TileContext(nc) as tc:
        with (
            tc.tile_pool(name="consts", bufs=1) as consts,
            tc.tile_pool(name="emi", bufs=2) as emi_pool,
            tc.tile_pool(name="xi", bufs=2) as xi_pool,
            tc.tile_pool(name="emc", bufs=2) as emc_pool,
            tc.tile_pool(name="x4", bufs=3) as x4_pool,
            tc.tile_pool(name="pst", bufs=3) as pst_pool,
            tc.tile_pool(name="oh", bufs=16) as oh_pool,
            tc.tile_pool(name="num", bufs=2) as num_pool,
            tc.tile_pool(name="scr", bufs=1) as scr_pool,
            tc.tile_pool(name="small", bufs=2) as small_pool,
            tc.tile_pool(name="psx", bufs=3, space="PSUM") as psx_pool,
            tc.tile_pool(name="pss", bufs=2, space="PSUM") as pss_pool,
            tc.tile_pool(name="psj", bufs=1, space="PSUM") as psj_pool,
        ):
            # ---------------- constants ----------------
            ident_b = consts.tile([128, 128], bf16)
            make_identity(nc, ident_b)
            ident_f = consts.tile([NT, NT], f32)
            make_identity(nc, ident_f)

            t_sb = consts.tile([NT, NT], f32)
            nc.sync.dma_start(out=t_sb, in_=trans_d)
            # start/end padded to 64 rows (rows 48:64 = 0 -> exp = 1) so
            # the init ACT copies cover the junk rows with finite data.
            start_sb = consts.tile([BO, 1], f32)
            nc.vector.memset(start_sb, 0.0)
            nc.sync.dma_start(out=start_sb[0:NT, :], in_=start_d)
            end_sb = consts.tile([BO, 1], f32)
            nc.vector.memset(end_sb, 0.0)
            nc.sync.dma_start(out=end_sb[0:NT, :], in_=end_d)

            exp_st = consts.tile([BO, 1], f32)
            nc.scalar.activation(exp_st, start_sb, AF.Exp)
            exp_en = consts.tile([BO, 1], f32)
            nc.scalar.activation(exp_en, end_sb, AF.Exp)

            ones48 = consts.tile([NT, 1], f32)
            nc.vector.memset(ones48, 1.0)
            nls = consts.tile([NT, 1], f32)
            nc.vector.memset(nls, -LOG_SCALE)

            # W = blockdiag(E @0, 0, E^T @64), E = exp(T - LOG_SCALE), bf16
            w_sb = consts.tile([H, H], bf16)
            nc.vector.memset(w_sb, 0.0)
            ps_tt = psj_pool.tile([NT, NT], f32)
            nc.tensor.transpose(ps_tt, t_sb, ident_f)
            nc.scalar.activation(w_sb[0:NT, 0:NT], t_sb, AF.Exp, bias=nls[:, 0:1])
            nc.scalar.activation(
                w_sb[BO : BO + NT, BO : BO + NT], ps_tt, AF.Exp, bias=nls[:, 0:1]
            )

            data_ones = consts.tile([B, SC], bf16)
            nc.vector.memset(data_ones, 1.0)

            idx16 = consts.tile([B, S], i16)
            nc.sync.dma_start(out=idx16, in_=sidx_d)

            # pair-count matrix chunks (contraction rows on partitions) and
            # the flat [T | start | end | pad] value vector, column per chunk
            ct_sb = consts.tile([128, KCT], bf16)
            for k in range(NKC):
                nc.sync.dma_start(
                    out=ct_sb[:, 128 * k : 128 * (k + 1)],
                    in_=ct_d[128 * k : 128 * (k + 1), :],
                )
            tf_f = consts.tile([128, NKC], f32)
            nc.sync.dma_start(
                out=tf_f[:, 0 : NKC - 1],
                in_=bass.AP(
                    tensor=trans_d.tensor,
                    offset=trans_d.offset,
                    ap=[[1, 128], [128, NKC - 1]],
                ),
            )
            nc.vector.memset(tf_f[:, NKC - 1 : NKC], 0.0)
            nc.sync.dma_start(
                out=tf_f[0:NT, NKC - 1 : NKC],
                in_=bass.AP(
                    tensor=start_d.tensor, offset=start_d.offset, ap=[[1, NT]]
                ),
            )
            nc.sync.dma_start(
                out=tf_f[NT : 2 * NT, NKC - 1 : NKC],
                in_=bass.AP(
                    tensor=end_d.tensor, offset=end_d.offset, ap=[[1, NT]]
                ),
            )
            tf_sb = consts.tile([128, NKC], bf16)
            nc.scalar.activation(tf_sb, tf_f, AF.Copy)

            # ---------------- chunked loads; exp is paced lazily ---------
            emi_tiles = {}
            xi_tiles = {}
            for c in range(NCH):
                emt = emi_pool.tile([B, CH * H], bf16, tag="emi")
                nc.sync.dma_start(
                    out=emt, in_=emi_d[:, H * CH * c : H * CH * (c + 1)]
                )
                emi_tiles[c] = emt
                xi_tiles[c] = xi_pool.tile([B, CH * H], bf16, tag="xi", name=f"xi{c}")

            def exp_piece(p):
                """exp 16 scan steps (2048 cols) of chunk p//4."""
                c, q = p // 4, p % 4
                nc.scalar.activation(
                    xi_tiles[c][:, 2048 * q : 2048 * (q + 1)],
                    emi_tiles[c][:, 2048 * q : 2048 * (q + 1)],
                    AF.Exp,
                )

            for p in range(3):
                exp_piece(p)

            emc_tiles = {}
            for c in range(S // CC):
                emt = emc_pool.tile([B, CC * NT], bf16, tag="emc")
                nc.sync.dma_start(
                    out=emt, in_=emc_d[:, NT * CC * c : NT * CC * (c + 1)]
                )
                emc_tiles[c] = emt

            # ---------------- numerator: scatters (gpsimd, early) --------
            ld_ls = nc.gpsimd.load_library(library_config.local_scatter)
            scatter_insts = []
            oh_tiles = []
            for k in range(S // SC):
                oh = oh_pool.tile([B, SC * NT], bf16, tag="oh")
                sc_i = nc.gpsimd.local_scatter(
                    out_ap=oh,
                    data_ap=data_ones,
                    idxs_ap=idx16[:, SC * k : SC * (k + 1)],
                    channels=B,
                    num_elems=SC * NT,
                    num_idxs=SC,
                )
                add_dep_helper(sc_i.ins, ld_ls.ins, reason="lib order")
                scatter_insts.append(sc_i)
                oh_tiles.append(oh)

            reds_em = consts.tile([B, S // SC], f32)

            def em_gold_piece(k):
                """DVE mul + ACT accum-reduce for scatter chunk k."""
                c, hh = k // 4, k % 4
                prod = num_pool.tile([B, SC * NT], bf16, tag="prod")
                nc.vector.tensor_mul(
                    prod,
                    emc_tiles[c][:, SC * NT * hh : SC * NT * (hh + 1)],
                    oh_tiles[k],
                )
                scr = scr_pool.tile([B, SC * NT], bf16, tag="scr")
                nc.scalar.activation(
                    scr, prod, AF.Copy, accum_out=reds_em[:, k : k + 1]
                )

            # ---------------- scan ----------------
            def xpose_group(g):
                """Transpose x for scan steps 4g..4g+3 into one PSUM tile,
                then one ACT copy to SBUF bf16."""
                ps_x = psx_pool.tile([128, GRP * 128], bf16, tag="psx")
                for sl in range(GRP):
                    s = GRP * g + sl
                    c, tf = s // CH, s % CH
                    nc.tensor.matmul(
                        ps_x[:, 128 * sl : 128 * (sl + 1)],
                        xi_tiles[c][:, H * tf : H * (tf + 1)],
                        ident_b,
                        is_transpose=True,
                        start=True,
                        stop=True,
                        skip_group_check=True,
                    )
                x4 = x4_pool.tile([128, GRP * 128], bf16, tag="x4")
                nc.scalar.activation(x4, ps_x, AF.Copy)
                return x4

            # init step s=0: p0 = x_0 * exp(start) (fwd), x_511 * exp(end)
            x4_tiles = [xpose_group(0), xpose_group(1)]
            p_state = pst_pool.tile([H, B], bf16, tag="pst")
            nc.scalar.activation(
                p_state[0:BO, :], x4_tiles[0][0:BO, 0:128], AF.Copy,
                scale=exp_st[:, 0:1],
            )
            nc.scalar.activation(
                p_state[BO:H, :], x4_tiles[0][BO:H, 0:128], AF.Copy,
                scale=exp_en[:, 0:1],
            )

            # main scan: s = 1..255, with numerator work sprinkled in.
            # Transpose groups are emitted AFTER the chain ops so the W
            # matmul wins PE priority ties; x4/psx buf rotation gives the
            # lookahead window.
            for s in range(1, NS):
                x4_cur = x4_tiles[s // GRP]
                ps_s = pss_pool.tile([H, B], f32, tag="pss")
                nc.tensor.matmul(ps_s, w_sb, p_state, start=True, stop=True)
                p_new = pst_pool.tile([H, B], bf16, tag="pst")
                nc.vector.tensor_mul(
                    p_new, ps_s, x4_cur[:, 128 * (s % GRP) : 128 * (s % GRP + 1)]
                )
                p_state = p_new
                if s % GRP == 1 and s // GRP + 2 < NS // GRP:
                    x4_tiles.append(xpose_group(s // GRP + 2))
                if s % 16 == 2 and s // 16 + 3 < 16:
                    exp_piece(s // 16 + 3)
                # numerator DVE/ACT pieces, paced into the second half of
                # the scan (their gpsimd producers are all done by then)
                if 112 <= s < 112 + 4 * 16 and (s - 112) % 4 == 0:
                    em_gold_piece((s - 112) // 4)

            # ---------------- final gold assembly ----------------
            em_gold = small_pool.tile([B, 1], f32)
            nc.vector.tensor_reduce(
                out=em_gold, in_=reds_em, axis=mybir.AxisListType.X, op=ALU.add
            )
            nc.sync.dma_start(out=gold_d[:, 0:1], in_=em_gold)
            # rest_gold[b] = sum_k C_b[k] * Tflat[k]: 19 accumulating matmuls
            ps_c = psj_pool.tile([B, 1], f32)
            for k in range(NKC):
                nc.tensor.matmul(
                    ps_c,
                    ct_sb[:, 128 * k : 128 * (k + 1)],
                    tf_sb[:, k : k + 1],
                    start=(k == 0),
                    stop=(k == NKC - 1),
                )
            rest_col = small_pool.tile([B, 1], f32)
            nc.scalar.activation(rest_col, ps_c, AF.Copy)
            nc.sync.dma_start(out=gold_d[:, 1:2], in_=rest_col)

            # ---------------- join ----------------
            ps_j = pss_pool.tile([H, B], f32, tag="pss")
            nc.tensor.matmul(ps_j, w_sb, p_state, start=True, stop=True)
            r_sb = small_pool.tile([NT, B], f32)
            nc.scalar.activation(r_sb, p_state[BO : BO + NT, :], AF.Copy)
            jprod = small_pool.tile([NT, B], f32)
            nc.vector.tensor_mul(jprod, ps_j[0:NT, :], r_sb)
            ps_z = psj_pool.tile([1, B], f32)
            nc.tensor.matmul(ps_z, ones48, jprod, start=True, stop=True)
            zlog_sb = small_pool.tile([1, B], f32)
            nc.scalar.activation(zlog_sb, ps_z, AF.Ln)
            nc.sync.dma_start(out=zlog_d, in_=zlog_sb)

    nc.compile()
    return nc


def _get_nc():
    if "nc" not in _CACHE:
        _CACHE["nc"] = _build()
    return _CACHE["nc"]


def make_indices(tg):
    """Host-side tag bookkeeping: scatter indices + pair-count matrix."""
    import ml_dtypes

    Bc = tg.shape[0]
    t_ar = np.arange(S)
    sidx = (NT * (t_ar % SC)[None, :] + tg).astype(np.int16)

    pair = NT * tg[:, :-1] + tg[:, 1:]  # (Bc, S-1)
    bi = np.repeat(np.arange(Bc), S - 1)
    flat = bi * KCT + pair.reshape(-1)
    ct = np.bincount(flat, minlength=Bc * KCT).reshape(Bc, KCT)
    ct[np.arange(Bc), NT * NT + tg[:, 0]] += 1
    ct[np.arange(Bc), NT * NT + NT + tg[:, -1]] += 1
    ct = np.ascontiguousarray(ct.T.astype(ml_dtypes.bfloat16))  # (KCT, Bc)
    return sidx, ct


def make_in_maps(emissions, tags, transitions, start_transitions, end_transitions):
    import ml_dtypes

    bf16 = ml_dtypes.bfloat16
    em = np.asarray(emissions, dtype=np.float32).reshape(NCORES * B, S, NT)
    tg = np.ascontiguousarray(np.asarray(tags).astype(np.int64))
    tr = np.ascontiguousarray(np.asarray(transitions, dtype=np.float32))
    st = np.ascontiguousarray(np.asarray(start_transitions, dtype=np.float32))
    en = np.ascontiguousarray(np.asarray(end_transitions, dtype=np.float32))

    # interleaved scan layout: [fwd em_s | pad16 | bwd em_{511-s} | pad16]
    emi = np.zeros((NCORES * B, NS, H), dtype=bf16)
    emi[:, :, 0:NT] = em[:, :NS, :]
    emi[:, :, BO : BO + NT] = em[:, : NS - 1 : -1, :]
    emi = emi.reshape(NCORES * B, NS * H)
    emc = np.ascontiguousarray(em.reshape(NCORES * B, S * NT).astype(bf16))

    in_maps = []
    for c in range(NCORES):
        sl = slice(c * B, (c + 1) * B)
        sidx, ct = make_indices(tg[sl])
        in_maps.append(
            {
                "emi": np.ascontiguousarray(emi[sl]),
                "emc": emc[sl],
                "sidx": sidx,
                "ct": ct,
                "trans": tr,
                "start_t": st,
                "end_t": en,
            }
        )
    return in_maps


def kernel(emissions, tags, mask, transitions, start_transitions, end_transitions):
    from concourse.bass_utils import run_bass_kernel_spmd

    nc = _get_nc()
    in_maps = make_in_maps(
        emissions, tags, transitions, start_transitions, end_transitions
    )
    res = run_bass_kernel_spmd(nc, in_maps, core_ids=list(range(NCORES)))

    total = 0.0
    for r in res.results:
        logz = r["zlog"].astype(np.float64)[0] + (S - 1) * LOG_SCALE
        gold = r["gold"].astype(np.float64)
        total += (logz - gold[:, 0] - gold[:, 1]).sum()
    loss = total / (NCORES * B)
    return np.asarray(loss, dtype=np.float32)
